# revision 19
# baseline (speedup 1.0000x reference)
"""CDFNormalizer (histogram binning) Trainium2 Bass kernel — additive-step v2.

z[n,d] = LUT[searchsorted(quantiles[:,d], x[n,d], side='left')]
with LUT[j] = sqrt(2)*erfinv(2*clip(j/1023, eps, 1-eps)-1).

Device model (per dim d, dim-major layout, fully additive):

  t   = fp32(x*inv + shift)                     (ACT, per-partition scale/bias)
  tc  = clamp(t, -1, 1)                         (DVE)
  h   = P_d(tc)                                 (DVE Horner, fp32)
  z   = h + sum_k w_k * [tb > v_k]              (bulk knots, bf16 compare on
                                                 tb = bf16(t), DVE)
          + sum_j D_j * [t > c_j]               (exact tail steps: ACT Sign /
                                                 Pool fp32 compare)

All step tiles and h are accumulated in a dim-major PSUM tile by PE matmuls
(lhsT = diag weight matrix, rhs = step tile, 512-wide, PSUM accumulate), then
transposed back to row-major by PE and DMA'd out.  P is constrained so the
tail steps telescope the exact LUT values at the distribution edges.

Data-parallel across 8 NeuronCores along the row axis.
"""

import math

import numpy as np

N = 2_097_152
D = 32
BINS = 1024
EPS = 1e-06
SQRT2 = 1.41421356
NCORES = 8
RPC = N // NCORES

TILE_ROWS = 8192
TFREE = 2048
NBLK = TFREE // 128          # 16 transpose blocks per tile
NGRP = TFREE // 512          # 4 psum accumulation groups per tile
NTILES = RPC // TILE_ROWS

# --- model structure (tuned) ---
DEG = 6                      # poly degree
NK = 14                      # bulk knots (DVE bf16 steps)
KL = 6                       # exact left tail steps
KR = 6                       # exact right tail steps
NTAIL = KL + KR
N_TAIL_ACT = 8               # tails produced by ACT Sign (rest: Pool)
N_TAIL_POOL = NTAIL - N_TAIL_ACT
POOL_CHAIN = True            # chain Pool tail tiles into one PE term
DVE_PAIRS = 0                # pairs of bulk step tiles merged on DVE
IO_F32R = False               # use float32r for in/out transposes + h term

# consts column layout
COL_INV = 0
COL_SHIFT = 1
COL_A = 2                    # A, a1..a_{DEG-1}, c0  (DEG+1 cols)
COL_C0 = COL_A + DEG
COL_KT = COL_C0 + 1          # NK knot thresholds (fp32 value of bf16 point)
COL_KW = COL_KT + NK         # NK knot weights
COL_TT = COL_KW + NK         # NTAIL tail thresholds (t-space)
COL_TB = COL_TT + NTAIL      # NTAIL negated tail thresholds (ACT bias)
COL_TD = COL_TB + NTAIL      # NTAIL tail deltas (bf16-rounded, fp32 stored)
NCONST = COL_TD + NTAIL


def _erfinv(y: float) -> float:
    if y <= -1.0:
        return -math.inf
    if y >= 1.0:
        return math.inf
    w = -math.log((1.0 - y) * (1.0 + y))
    if w < 5.0:
        w2 = w - 2.5
        p = 2.81022636e-08
        for c in (3.43273939e-07, -3.5233877e-06, -4.39150654e-06, 2.1858087e-04,
                  -1.25372503e-03, -4.17768164e-03, 2.46640727e-01, 1.50140941e00):
            p = p * w2 + c
        x = p * y
    else:
        w2 = math.sqrt(w) - 3.0
        p = -2.00214257e-04
        for c in (1.00950558e-04, 1.34934322e-03, -3.67342844e-03, 5.73950773e-03,
                  -7.62246130e-03, 9.43887047e-03, 1.00167406e00, 2.83297682e00):
            p = p * w2 + c
        x = p * y
    c2 = 2.0 / math.sqrt(math.pi)
    for _ in range(3):
        err = math.erf(x) - y
        x -= err / (c2 * math.exp(-x * x))
    return x


def _build_lut() -> np.ndarray:
    j = np.arange(BINS + 1, dtype=np.float64)
    u = np.clip(j / (BINS - 1), EPS, 1.0 - EPS)
    lut = np.array([_erfinv(2.0 * ui - 1.0) for ui in u], dtype=np.float64)
    return lut * SQRT2


def _bf16_boundary_above(v32):
    """Comparing bf16(t) > v (v a bf16 grid point, fp32) equals t32 > m with m
    the RNE rounding boundary just above v."""
    import ml_dtypes
    bf = ml_dtypes.bfloat16
    v = np.asarray(v32, dtype=np.float64)
    bv = v.astype(np.float32).astype(bf)
    nb = np.where(bv.astype(np.float64) > v, bv, np.nextafter(bv, np.inf)).astype(bf)
    pb = np.nextafter(nb, -np.inf)
    return 0.5 * (nb.astype(np.float64) + pb.astype(np.float64))


def _fit_dim(qd: np.ndarray, lutd: np.ndarray) -> dict:
    """Constrained poly + greedy bf16 knots; exact tail steps."""
    import ml_dtypes
    bf = ml_dtypes.bfloat16

    lo_x, hi_x = qd[KL - 1], qd[BINS - KR]
    mu = 0.5 * (lo_x + hi_x)
    inv = 2.0 / (hi_x - lo_x)
    inv32 = np.float32(inv)
    shift32 = np.float32(-mu * inv)

    def t32_of(x):
        return (np.asarray(x, np.float64) * np.float64(inv32)
                + np.float64(shift32)).astype(np.float32).astype(np.float64)

    bs = np.arange(KL, BINS - KR + 1)
    xm = 0.5 * (qd[bs - 1] + qd[bs])
    ym = lutd[bs]
    tm = np.clip(t32_of(xm), -1.0, 1.0)
    tm_raw = t32_of(xm)
    nb = len(bs)
    V = np.vander(tm, DEG + 1, increasing=True)
    e0 = np.vander([-1.0], DEG + 1, increasing=True)[0]
    e1 = np.vander([1.0], DEG + 1, increasing=True)[0]
    yL = lutd[KL]
    yH = lutd[BINS - KR]

    def solve(step_cols, wfix=None):
        if wfix is None:
            S = np.stack(step_cols, 1) if step_cols else np.zeros((nb, 0))
            X = np.concatenate([V, S], 1)
            nv = X.shape[1]
            C = np.zeros((2, nv))
            C[0, :DEG + 1] = e0
            C[1, :DEG + 1] = e1
            C[1, DEG + 1:] = 1.0
            d = np.array([yL, yH])
            y = ym
        else:
            y = ym.copy()
            for col, w in zip(step_cols, wfix):
                y = y - w * col
            X = V
            nv = DEG + 1
            C = np.stack([e0, e1])
            d = np.array([yL, yH - float(np.sum(wfix))])
        A = X.T @ X
        b = X.T @ y
        K = np.block([[A, C.T], [C, np.zeros((2, 2))]])
        sol = np.linalg.lstsq(K, np.concatenate([b, d]), rcond=None)[0]
        beta = sol[:nv]
        return beta, y - X @ beta

    knot_bins: list[int] = []
    cols: list[np.ndarray] = []
    beta, r = solve(cols)
    for _ in range(NK):
        csum = np.cumsum(r[::-1])[::-1]
        cnt = np.arange(nb, 0, -1)
        gain = np.zeros(nb)
        gain[1:] = csum[1:] ** 2 / cnt[1:]
        for jb in knot_bins:
            i = jb - KL + 1
            gain[max(0, i - 1):i + 2] = 0
        i_star = int(np.argmax(gain))
        j_star = int(bs[i_star] - 1)
        knot_bins.append(j_star)
        cols.append((bs > j_star).astype(np.float64))
        beta, r = solve(cols)

    # snap knot thresholds to bf16 grid; refit with effective columns
    thr_t = t32_of(qd[knot_bins]).astype(np.float32)
    v_b16 = thr_t.astype(bf).astype(np.float32)      # compare scalar on device
    m_eval = _bf16_boundary_above(thr_t)
    cols_eff = [(tm_raw > m).astype(np.float64) for m in m_eval]
    beta2, _ = solve(cols_eff)
    w_b16 = np.asarray(beta2[DEG + 1:], np.float32).astype(bf).astype(np.float64)
    beta3, _ = solve(cols_eff, wfix=w_b16)
    coeffs = beta3  # c0..cDEG (fp64; rounded later)

    # tails: t-space thresholds + bf16 deltas
    tail_c, tail_d = [], []
    for j in list(range(KL)) + list(range(BINS - KR, BINS)):
        tail_c.append(np.float32(t32_of(qd[j])))
        tail_d.append(float(lutd[j + 1] - lutd[j]))
    # shift: left tails telescope from lut[0]
    c0 = coeffs[0] - (lutd[KL] - lutd[0])

    # ACT-assigned tails use Sign: contribution D/2*sign + D/2
    d_half = [float(np.float32(d / 2.0).astype(bf)) for d in tail_d]
    d_full = [float(np.float32(d).astype(bf)) for d in tail_d]
    for j in range(N_TAIL_ACT):
        c0 = c0 + d_half[j]

    return {
        "inv": float(inv32), "shift": float(shift32),
        "A": coeffs[DEG], "a": [coeffs[DEG - i] for i in range(1, DEG)],
        "c0": c0,
        "kt": list(v_b16), "kw": list(w_b16.astype(np.float64)),
        "tc": [float(c) for c in tail_c],
        "td_half": d_half, "td_full": d_full,
    }


def _build_consts(quantiles: np.ndarray) -> dict:
    import ml_dtypes
    bf = ml_dtypes.bfloat16

    lutd = _build_lut()
    fits = [_fit_dim(quantiles[:, d].astype(np.float64), lutd) for d in range(D)]
    cols = []

    def col(vals):
        cols.append(np.asarray(vals, dtype=np.float64))

    col([f["inv"] for f in fits])
    col([f["shift"] for f in fits])
    col([f["A"] for f in fits])
    for i in range(DEG - 1):
        col([f["a"][i] for f in fits])
    col([f["c0"] for f in fits])
    for k in range(NK):
        col([f["kt"][k] for f in fits])
    for k in range(NK):
        col([f["kw"][k] for f in fits])
    for j in range(NTAIL):
        col([f["tc"][j] for f in fits])
    for j in range(NTAIL):
        col([-f["tc"][j] for f in fits])
    for j in range(NTAIL):
        col([f["td_full"][j] for f in fits])
    consts32 = np.stack(cols, axis=1)
    consts = np.tile(consts32, (4, 1)).astype(np.float32)  # partition q -> d=q%32

    identf = np.eye(128, dtype=np.float32)
    # bf16 matmul weight matrices: ident + one diag per ACT tail
    mats = [np.eye(128, dtype=np.float32)]
    for j in range(N_TAIL_ACT):
        dia = np.zeros((128, 128), dtype=np.float32)
        for q in range(128):
            dia[q, q] = fits[q % D]["td_half"][j]
        mats.append(dia)
    matsb = np.concatenate(mats, axis=1).astype(bf)

    return {"consts": consts, "identf": identf, "matsb": matsb}


def build_kernel(rpc: int = RPC, ntiles: int | None = None,
                 finalize: bool = True, repeat: int = 1):
    import concourse.bass as bass
    import concourse.mybir as mybir
    from concourse import bacc, tile

    if ntiles is None:
        ntiles = rpc // TILE_ROWS
    dt = mybir.dt.float32
    bf = mybir.dt.bfloat16
    f32r = mybir.dt.float32r
    op = mybir.AluOpType
    AF = mybir.ActivationFunctionType

    nc = bacc.Bacc(None)
    x_ext = nc.declare_dram_parameter("x", [rpc, D], dt, isOutput=False)
    consts_ext = nc.declare_dram_parameter("consts", [128, NCONST], dt,
                                           isOutput=False)
    identf_ext = nc.declare_dram_parameter("identf", [128, 128], dt,
                                           isOutput=False)
    matsb_ext = nc.declare_dram_parameter("matsb", [128, 128 * (1 + N_TAIL_ACT)],
                                          bf, isOutput=False)
    z_ext = nc.declare_dram_parameter("z", [rpc, D], dt, isOutput=True)

    x_view = x_ext.rearrange("(p g) d -> p (g d)", p=128)
    z_view = z_ext.rearrange("(p g) d -> p (g d)", p=128)
    gpt = TFREE

    with tile.TileContext(nc) as tc:
        with (
            tc.tile_pool(name="const", bufs=1) as cpool,
            tc.tile_pool(name="work", bufs=2) as wpool,
            tc.tile_pool(name="steps", bufs=1) as spool,
            tc.tile_pool(name="zw", bufs=2) as zpool,
            tc.tile_pool(name="pin", bufs=1, space="PSUM") as pin,
            tc.tile_pool(name="pacc", bufs=1, space="PSUM") as pac,
        ):
            ct = cpool.tile([128, NCONST], dt, tag="consts")
            identf = cpool.tile([128, 128], dt, tag="identf")
            matsb = cpool.tile([128, 128 * (1 + N_TAIL_ACT)], bf, tag="matsb")
            nc.sync.dma_start(ct[:], consts_ext[:])
            nc.sync.dma_start(identf[:], identf_ext[:])
            nc.sync.dma_start(matsb[:], matsb_ext[:])

            def sc(j):
                return ct[:, j:j + 1]

            identb = matsb[:, 0:128]

            def diag_act(j):
                return matsb[:, (1 + j) * 128:(2 + j) * 128]

            def drain(pzA, pzB, it):
                """Drain tile `it`'s accumulated dim-major psum to DRAM."""
                zdmA = zpool.tile([128, TFREE // 2], dt, tag="zdmA")
                nc.scalar.copy(zdmA[:], pzA[:])
                zdmB = zpool.tile([128, TFREE // 2], dt, tag="zdmB")
                nc.scalar.copy(zdmB[:], pzB[:])
                zrm = pin.tile([128, TFREE], dt, tag="xt")
                for k in range(NBLK):
                    src = zdmA if k < NBLK // 2 else zdmB
                    kk = k % (NBLK // 2)
                    if IO_F32R:
                        nc.tensor.matmul(
                            zrm[:, k * 128:(k + 1) * 128].bitcast(f32r),
                            src[:, kk * 128:(kk + 1) * 128].bitcast(f32r),
                            identf[:].bitcast(f32r), is_transpose=True)
                    else:
                        nc.tensor.transpose(zrm[:, k * 128:(k + 1) * 128],
                                            src[:, kk * 128:(kk + 1) * 128],
                                            identf[:])
                zs = zpool.tile([128, TFREE], dt, tag="zs")
                nc.scalar.copy(zs[:], zrm[:])
                nc.sync.dma_start(z_view[:, it * gpt:(it + 1) * gpt], zs[:])

            def prologue(it):
                """Next tile's input path: DMA, in-transposes, ACT t."""
                xn = wpool.tile([128, TFREE], dt, tag="xn")
                nc.sync.dma_start(xn[:], x_view[:, it * gpt:(it + 1) * gpt])
                xtp = pin.tile([128, TFREE], dt, tag="xt")
                for k in range(NBLK):
                    if IO_F32R:
                        nc.tensor.matmul(
                            xtp[:, k * 128:(k + 1) * 128].bitcast(f32r),
                            xn[:, k * 128:(k + 1) * 128].bitcast(f32r),
                            identf[:].bitcast(f32r), is_transpose=True)
                    else:
                        nc.tensor.transpose(xtp[:, k * 128:(k + 1) * 128],
                                            xn[:, k * 128:(k + 1) * 128],
                                            identf[:])
                t = wpool.tile([128, TFREE], dt, tag="t")
                nc.scalar.activation(t[:], xtp[:], AF.Identity,
                                     bias=sc(COL_SHIFT), scale=sc(COL_INV))
                return t, xtp

            tile_seq = [it for _rep in range(repeat) for it in range(ntiles)]
            t_next, xtp_next = prologue(tile_seq[0])
            for seq_i, it in enumerate(tile_seq):
                t = t_next
                xtp = xtp_next

                tb = wpool.tile([128, TFREE], bf, tag="tb")
                nc.vector.tensor_copy(tb[:], t[:])

                pzA = pac.tile([128, TFREE // 2], dt, tag="zacc", bufs=2)
                pzB = pac.tile([128, TFREE // 2], dt, tag="zacc", bufs=2)
                started = [False] * NGRP

                def term(src_ap, wmat, last=False, cast_f32r=False):
                    for g in range(NGRP):
                        lhsT = wmat
                        rhs = src_ap[:, g * 512:(g + 1) * 512]
                        ph = pzA if g < NGRP // 2 else pzB
                        c0 = (g % (NGRP // 2)) * 512
                        out = ph[:, c0:c0 + 512]
                        if cast_f32r:
                            lhsT = lhsT.bitcast(f32r)
                            rhs = rhs.bitcast(f32r)
                        nc.tensor.matmul(out, lhsT, rhs,
                                         start=not started[g], stop=last)
                        started[g] = True

                # bulk knots (DVE, weighted at produce)
                pair_buf = None
                for k in range(NK):
                    s = spool.tile([128, TFREE], bf, tag="sd", bufs=8)
                    nc.vector.tensor_scalar(s[:], tb[:], sc(COL_KT + k),
                                            sc(COL_KW + k), op.is_gt, op.mult)
                    if k < 2 * DVE_PAIRS:
                        if pair_buf is None:
                            pair_buf = s
                        else:
                            m = spool.tile([128, TFREE], bf, tag="sdm", bufs=2)
                            nc.vector.tensor_tensor(m[:], pair_buf[:], s[:],
                                                    op.add)
                            term(m[:], identb)
                            pair_buf = None
                    else:
                        term(s[:], identb)

                # ACT Sign tails
                for j in range(N_TAIL_ACT):
                    sa = spool.tile([128, TFREE], bf, tag="sa", bufs=8)
                    nc.scalar.activation(sa[:], t[:], AF.Sign,
                                         bias=sc(COL_TB + j), scale=1.0)
                    term(sa[:], diag_act(j))

                # Pool tails (weighted at produce), optionally chained
                pchain = None
                for jj in range(N_TAIL_POOL):
                    j = N_TAIL_ACT + jj
                    sp = spool.tile([128, TFREE], bf, tag="sp", bufs=3)
                    nc.gpsimd.tensor_scalar(sp[:], t[:], sc(COL_TT + j),
                                            sc(COL_TD + j), op.is_gt, op.mult)
                    if POOL_CHAIN:
                        if pchain is None:
                            pchain = sp
                        else:
                            m = spool.tile([128, TFREE], bf, tag="spm", bufs=2)
                            nc.gpsimd.tensor_tensor(m[:], pchain[:], sp[:],
                                                    op.add)
                            term(m[:], identb)
                            pchain = None
                    else:
                        term(sp[:], identb)
                if POOL_CHAIN and pchain is not None:
                    term(pchain[:], identb)

                # next tile's input path
                if seq_i + 1 < len(tile_seq):
                    t_next, xtp_next = prologue(tile_seq[seq_i + 1])

                # poly (DVE Horner) -> fp32 term, closes accumulation
                tc_t = wpool.tile([128, TFREE], dt, tag="tc")
                nc.vector.tensor_scalar(tc_t[:], t[:], -1.0, 1.0,
                                        op.max, op.min)
                h = wpool.tile([128, TFREE], dt, tag="h")
                nc.vector.tensor_scalar(h[:], tc_t[:], sc(COL_A), None, op.mult)
                for i in range(DEG - 1):
                    nc.vector.scalar_tensor_tensor(h[:], h[:], sc(COL_A + 1 + i),
                                                   tc_t[:], op.add, op.mult)
                nc.vector.tensor_scalar(h[:], h[:], sc(COL_C0), None, op.add)
                term(h[:], identf[:], last=True, cast_f32r=IO_F32R)
                drain(pzA, pzB, it)

    if finalize:
        nc.finalize()
    return nc


_CACHE: dict = {}


def kernel(x: np.ndarray, quantiles: np.ndarray) -> np.ndarray:
    from concourse.bass_utils import run_bass_kernel_spmd

    x = np.ascontiguousarray(np.asarray(x, dtype=np.float32))
    quantiles = np.ascontiguousarray(np.asarray(quantiles, dtype=np.float32))
    assert x.shape == (N, D) and quantiles.shape == (BINS, D)

    consts = _build_consts(quantiles)
    if "nc" not in _CACHE:
        _CACHE["nc"] = build_kernel()
    nc = _CACHE["nc"]

    core_ids = list(range(NCORES))
    in_maps = [
        {"x": x[c * RPC:(c + 1) * RPC], **consts}
        for c in core_ids
    ]
    res = run_bass_kernel_spmd(nc, in_maps, core_ids)
    out = np.concatenate([res.results[i]["z"] for i in range(NCORES)], axis=0)
    return out.astype(np.float32)
